# revision 1
# baseline (speedup 1.0000x reference)
"""MoE gate (group-limited top-k routing) as a Bass/Tile kernel for 8 TRN2 cores.

Computes, per token:
  logits = hidden @ W            (fp32-exact, K=7168, E=256)
  scores = sigmoid(logits) + bias
  group-limited routing: top-2-sum per group of 32 -> top-4 groups of 8
  top-8 of masked scores, renormalized, * 2.5

Sharding: data-parallel over tokens (1024 tokens/core), W + bias replicated.

Matmul schemes:
  f16x3 (default): split x and W into fp16 hi/lo parts (x = hi + lo with both
    parts exact fp16), compute hi@Whi + hi@Wlo + lo@Whi with fp16 matmuls
    (1 cycle/row) accumulating in fp32 PSUM. The dropped lo@Wlo term is
    O(2^-22) relative — result matches fp32 matmul to ~1e-6. 33% less PE
    time than the hardware fp32 path (which runs at 4 cycles/row).
  f32: plain fp32 matmuls (exact, slower).
Hidden tiles [128T, 128K] are PE-transposed in fp32 (exact) to [128K, 128T]
via PSUM; the PSUM->SBUF copyback performs the hi/lo split on DVE.
"""

import sys

if "/opt/trn_rl_repo" not in sys.path:
    sys.path.insert(0, "/opt/trn_rl_repo")

import numpy as np

import concourse.bacc as bacc
import concourse.bass as bass
import concourse.mybir as mybir
import concourse.tile as tile
from concourse import bass_utils
from concourse.masks import make_identity

P = 128
TOP_K = 8
N_GROUP = 8
TOPK_GROUP = 4
SCALE = 2.5

N_CORES = 8
TOKENS = 8192
HIDDEN = 7168
EXPERTS = 256


def build_moe_gate(
    tokens_per_core=TOKENS // N_CORES,
    hidden=HIDDEN,
    n_experts=EXPERTS,
    scheme="f16x3",
):
    KC = hidden // P          # K-chunks of 128
    TT = tokens_per_core // P  # token tiles of 128
    GS = n_experts // N_GROUP  # experts per group
    BATCH = 8 if KC % 8 == 0 else 4   # transposes batched per PSUM copyback
    WB = 8 if KC % 8 == 0 else 4      # weight-prep chunk batch
    f32 = mybir.dt.float32
    f16 = mybir.dt.float16
    E2 = 2 * n_experts

    nc = bacc.Bacc("TRN2", target_bir_lowering=False, debug=False)
    hs = nc.dram_tensor(
        "hidden_states", [tokens_per_core, hidden], f32, kind="ExternalInput"
    ).ap()
    wk = nc.dram_tensor("kernel", [hidden, n_experts], f32, kind="ExternalInput").ap()
    bias = nc.dram_tensor(
        "e_score_correction_bias", [n_experts], f32, kind="ExternalInput"
    ).ap()
    out = nc.dram_tensor(
        "topk_out", [tokens_per_core, TOP_K], f32, kind="ExternalOutput"
    ).ap()

    with tile.TileContext(nc) as tc:
        with (
            tc.tile_pool(name="const", bufs=1) as cpool,
            tc.tile_pool(name="wstage", bufs=2) as wspool,
            tc.tile_pool(name="hload", bufs=3) as hpool,
            tc.tile_pool(name="ht", bufs=4) as htpool,
            tc.tile_pool(name="ptr", bufs=3, space="PSUM") as ptpool,
            tc.tile_pool(name="plog", bufs=2, space="PSUM") as plpool,
            tc.tile_pool(name="route", bufs=2) as rpool,
        ):
            identity = cpool.tile([P, P], f32)
            make_identity(nc, identity)

            # --- resident replicated weights ---
            if scheme == "f32r":
                f32r = mybir.dt.float32r
                wk_mm = cpool.tile([P, KC, n_experts], f32r)
                wk_view = wk.rearrange("(kc p) e -> p kc e", p=P)
                for wb in range(KC // WB):
                    wstage = wspool.tile([P, WB, n_experts], f32)
                    nc.sync.dma_start(
                        out=wstage, in_=wk_view[:, wb * WB : (wb + 1) * WB, :]
                    )
                    # rounds to f32r as required by the verifier
                    nc.vector.tensor_copy(
                        wk_mm[:, wb * WB : (wb + 1) * WB, :], wstage
                    )
            elif scheme == "f16x3":
                # wsplit[p, k, 0:E] = fp16 hi part of W chunk k,
                # wsplit[p, k, E:2E] = fp16 lo part (W - hi)
                wsplit = cpool.tile([P, KC, E2], f16)
                wk_view = wk.rearrange("(kc p) e -> p kc e", p=P)
                for wb in range(KC // WB):
                    wstage = wspool.tile([P, WB, n_experts], f32)
                    nc.sync.dma_start(
                        out=wstage, in_=wk_view[:, wb * WB : (wb + 1) * WB, :]
                    )
                    ws = slice(wb * WB, (wb + 1) * WB)
                    # hi parts for WB chunks in one op, then lo parts in one op
                    nc.vector.tensor_copy(wsplit[:, ws, :n_experts], wstage)
                    nc.vector.tensor_sub(
                        wsplit[:, ws, n_experts:], wstage,
                        wsplit[:, ws, :n_experts],
                    )
            else:
                wk_sb = cpool.tile([P, KC, n_experts], f32)
                nc.sync.dma_start(
                    out=wk_sb, in_=wk.rearrange("(kc p) e -> p kc e", p=P)
                )

            # bias is only needed by the first routing epilogue, well into the
            # run; load it after the weight-prep DMAs are queued
            bias_sb = cpool.tile([P, n_experts], f32)
            bias_bcast = bass.AP(
                tensor=bias.tensor, offset=bias.offset, ap=[[0, P]] + list(bias.ap)
            )
            nc.gpsimd.dma_start(out=bias_sb, in_=bias_bcast)

            for t in range(TT):
                htile = hpool.tile([P, hidden], f32)
                # one load slice per transpose batch so early batches only
                # wait for their own slice (and slices spread across queues)
                for l in range(KC // BATCH):
                    sl = slice(l * BATCH * P, (l + 1) * BATCH * P)
                    nc.sync.dma_start(
                        out=htile[:, sl], in_=hs[t * P : (t + 1) * P, sl]
                    )

                logits_ps = plpool.tile(
                    [P, E2 if scheme == "f16x3" else n_experts], f32
                )

                n_mm = 0
                total_mm = KC * (2 if scheme == "f16x3" else 1)
                for b in range(KC // BATCH):
                    tp = ptpool.tile([P, BATCH * P], f32)
                    for j in range(BATCH):
                        k = b * BATCH + j
                        nc.tensor.transpose(
                            tp[:, j * P : (j + 1) * P],
                            htile[:, k * P : (k + 1) * P],
                            identity,
                        )
                    if scheme == "f16x3":
                        # PSUM -> SBUF copyback doubles as the hi/lo split:
                        # hi on the (otherwise idle) scalar engine, lo on DVE
                        hiT = htpool.tile([P, BATCH * P], f16)
                        nc.scalar.activation(
                            hiT, tp, mybir.ActivationFunctionType.Copy
                        )
                        loT = htpool.tile([P, BATCH * P], f16)
                        nc.vector.tensor_sub(loT, tp, hiT)
                        for j in range(BATCH):
                            k = b * BATCH + j
                            nc.tensor.matmul(
                                logits_ps,
                                lhsT=hiT[:, j * P : (j + 1) * P],
                                rhs=wsplit[:, k, :],
                                start=(n_mm == 0),
                                stop=(n_mm == total_mm - 1),
                            )
                            n_mm += 1
                            nc.tensor.matmul(
                                logits_ps[:, :n_experts],
                                lhsT=loT[:, j * P : (j + 1) * P],
                                rhs=wsplit[:, k, :n_experts],
                                start=(n_mm == 0),
                                stop=(n_mm == total_mm - 1),
                            )
                            n_mm += 1
                    else:
                        hT = htpool.tile(
                            [P, BATCH * P],
                            mybir.dt.float32r if scheme == "f32r" else f32,
                        )
                        nc.vector.tensor_copy(hT, tp)
                        rhs_w = wk_mm if scheme == "f32r" else wk_sb
                        for j in range(BATCH):
                            k = b * BATCH + j
                            nc.tensor.matmul(
                                logits_ps,
                                lhsT=hT[:, j * P : (j + 1) * P],
                                rhs=rhs_w[:, k, :],
                                start=(n_mm == 0),
                                stop=(n_mm == total_mm - 1),
                            )
                            n_mm += 1

                # ---- routing epilogue (tokens on partitions) ----
                sc = rpool.tile([P, n_experts], f32)
                if scheme == "f16x3":
                    # combine hi and lo expert columns (one PSUM read per op),
                    # then sigmoid
                    half = rpool.tile([P, n_experts], f32)
                    nc.vector.tensor_copy(half, logits_ps[:, n_experts:])
                    pre = rpool.tile([P, n_experts], f32)
                    nc.vector.tensor_add(pre, logits_ps[:, :n_experts], half)
                    nc.scalar.activation(
                        sc, pre, mybir.ActivationFunctionType.Sigmoid
                    )
                else:
                    nc.scalar.activation(
                        sc, logits_ps, mybir.ActivationFunctionType.Sigmoid
                    )
                nc.vector.tensor_add(sc, sc, bias_sb)

                # top-2 sum per group of GS experts
                m8 = rpool.tile([P, N_GROUP * 8], f32)
                for g in range(N_GROUP):
                    nc.vector.max(
                        m8[:, g * 8 : (g + 1) * 8], sc[:, g * GS : (g + 1) * GS]
                    )
                m8v = m8.rearrange("p (g k) -> p g k", k=8)
                gsum = rpool.tile([P, N_GROUP], f32)
                nc.vector.tensor_add(gsum, m8v[:, :, 0], m8v[:, :, 1])

                # top-TOPK_GROUP groups -> per-group 0/1 mask via threshold
                gmax = rpool.tile([P, 8], f32)
                nc.vector.max(gmax, gsum)
                gmask = rpool.tile([P, N_GROUP], f32)
                nc.vector.tensor_scalar(
                    gmask,
                    gsum,
                    gmax[:, TOPK_GROUP - 1 : TOPK_GROUP],
                    None,
                    op0=mybir.AluOpType.is_ge,
                )

                # masked scores = sc * mask (0 where group dropped)
                masked = rpool.tile([P, n_experts], f32)
                nc.vector.tensor_mul(
                    masked.rearrange("p (g e) -> p g e", g=N_GROUP),
                    sc.rearrange("p (g e) -> p g e", g=N_GROUP),
                    gmask[:, :, None].broadcast_to([P, N_GROUP, GS]),
                )

                top8 = rpool.tile([P, TOP_K], f32)
                nc.vector.max(top8, masked)

                dsum = rpool.tile([P, 1], f32)
                nc.vector.reduce_sum(dsum, top8, axis=mybir.AxisListType.X)
                rcp = rpool.tile([P, 1], f32)
                nc.vector.reciprocal(rcp, dsum)
                wout = rpool.tile([P, TOP_K], f32)
                nc.vector.tensor_scalar(
                    wout,
                    top8,
                    rcp,
                    SCALE,
                    op0=mybir.AluOpType.mult,
                    op1=mybir.AluOpType.mult,
                )
                nc.sync.dma_start(out=out[t * P : (t + 1) * P, :], in_=wout)

    nc.compile()
    return nc


_CACHE = {}


def _built_nc():
    if "nc" not in _CACHE:
        _CACHE["nc"] = build_moe_gate()
    return _CACHE["nc"]


def kernel(hidden_states, kernel, e_score_correction_bias):
    hs = np.ascontiguousarray(np.asarray(hidden_states), dtype=np.float32)
    wk = np.ascontiguousarray(np.asarray(kernel), dtype=np.float32)
    bi = np.ascontiguousarray(np.asarray(e_score_correction_bias), dtype=np.float32)
    assert hs.shape == (TOKENS, HIDDEN) and wk.shape == (HIDDEN, EXPERTS)

    tpc = TOKENS // N_CORES
    nc = _built_nc()
    in_maps = [
        {
            "hidden_states": hs[i * tpc : (i + 1) * tpc],
            "kernel": wk,
            "e_score_correction_bias": bi,
        }
        for i in range(N_CORES)
    ]
    res = bass_utils.run_bass_kernel_spmd(nc, in_maps, core_ids=list(range(N_CORES)))
    return np.concatenate(
        [res.results[i]["topk_out"] for i in range(N_CORES)], axis=0
    )



# revision 3
# speedup vs baseline: 1.6095x; 1.6095x over previous
"""MoE gate (group-limited top-k routing) as a Bass/Tile kernel for 8 TRN2 cores.

Computes, per token:
  logits = hidden @ W            (K=7168, E=256)
  scores = sigmoid(logits) + bias
  group-limited routing: top-2-sum per group of 32 -> top-4 groups of 8
  top-8 of masked scores, renormalized, * 2.5

Sharding: data-parallel over tokens (1024 tokens/core), W + bias replicated.

Matmul schemes:
  f32r1 (default): hidden tiles are PE-transposed in float32r (1.5 cyc/row)
    and the gating matmul streams W in float32r (1 cyc/row at 256 moving
    columns) -- one matmul per 128-K chunk instead of f16x3's three. All
    tensors stay plain fp32 in DRAM/SBUF; float32r is applied via bitcast
    views at the PE call sites only. ~19-bit effective mantissa.
  f16hi: fp32 PE transposes; the PSUM->SBUF copyback rounds to fp16; a
    single fp16 matmul per chunk (drops the x_lo correction term of the
    old f16x3 scheme; error ~2^-11 relative on logits, well within the
    2e-2 gate).
"""

import sys

if "/opt/trn_rl_repo" not in sys.path:
    sys.path.insert(0, "/opt/trn_rl_repo")

import numpy as np

import concourse.bacc as bacc
import concourse.bass as bass
import concourse.mybir as mybir
import concourse.tile as tile
from concourse import bass_utils
from concourse.masks import make_identity

P = 128
TOP_K = 8
N_GROUP = 8
TOPK_GROUP = 4
SCALE = 2.5

N_CORES = 8
TOKENS = 8192
HIDDEN = 7168
EXPERTS = 256

SCHEME = "f16hi"


def build_moe_gate(
    tokens_per_core=TOKENS // N_CORES,
    hidden=HIDDEN,
    n_experts=EXPERTS,
    scheme=SCHEME,
):
    KC = hidden // P          # K-chunks of 128
    TT = tokens_per_core // P  # token tiles of 128
    GS = n_experts // N_GROUP  # experts per group
    BATCH = 8                  # transposes batched per PSUM copyback
    WB = 8                     # weight-load chunk batch
    f32 = mybir.dt.float32
    f32r = mybir.dt.float32r
    f16 = mybir.dt.float16

    nc = bacc.Bacc("TRN2", target_bir_lowering=False, debug=False)
    hs = nc.dram_tensor(
        "hidden_states", [tokens_per_core, hidden], f32, kind="ExternalInput"
    ).ap()
    wk = nc.dram_tensor("kernel", [hidden, n_experts], f32, kind="ExternalInput").ap()
    bias = nc.dram_tensor(
        "e_score_correction_bias", [n_experts], f32, kind="ExternalInput"
    ).ap()
    out = nc.dram_tensor(
        "topk_out", [tokens_per_core, TOP_K], f32, kind="ExternalOutput"
    ).ap()

    with tile.TileContext(nc) as tc:
        with (
            tc.tile_pool(name="const", bufs=1) as cpool,
            tc.tile_pool(name="wstage", bufs=2) as wspool,
            tc.tile_pool(name="hload", bufs=3) as hpool,
            tc.tile_pool(name="ht", bufs=4) as htpool,
            tc.tile_pool(name="ptr", bufs=3, space="PSUM") as ptpool,
            tc.tile_pool(name="plog", bufs=2, space="PSUM") as plpool,
            tc.tile_pool(name="route", bufs=2) as rpool,
        ):
            identity = cpool.tile([P, P], f32)
            make_identity(nc, identity)

            # H tile 0 ahead of the weights so PE transposes start ASAP
            def load_htile(t):
                ht = hpool.tile([P, hidden], f32)
                # one load slice per transpose batch so early batches only
                # wait for their own slice (and slices spread across queues)
                for l in range(KC // BATCH):
                    sl = slice(l * BATCH * P, (l + 1) * BATCH * P)
                    nc.sync.dma_start(
                        out=ht[:, sl], in_=hs[t * P : (t + 1) * P, sl]
                    )
                return ht

            ht0 = load_htile(0)

            # --- resident replicated weights ---
            wk_view = wk.rearrange("(kc p) e -> p kc e", p=P)
            if scheme == "f32r1":
                wk_mm = cpool.tile([P, KC, n_experts], f32)
                for wb in range(KC // WB):
                    ws = slice(wb * WB, (wb + 1) * WB)
                    nc.sync.dma_start(out=wk_mm[:, ws, :], in_=wk_view[:, ws, :])
            elif scheme == "f16hi":
                wk_mm = cpool.tile([P, KC, n_experts], f16)
                for wb in range(KC // WB):
                    ws = slice(wb * WB, (wb + 1) * WB)
                    wstage = wspool.tile([P, WB, n_experts], f32)
                    nc.sync.dma_start(out=wstage, in_=wk_view[:, ws, :])
                    nc.vector.tensor_copy(wk_mm[:, ws, :], wstage)
            else:
                raise ValueError(scheme)

            # bias is only needed by the first routing epilogue, well into the
            # run; load it after the weight DMAs are queued
            bias_sb = cpool.tile([P, n_experts], f32)
            bias_bcast = bass.AP(
                tensor=bias.tensor, offset=bias.offset, ap=[[0, P]] + list(bias.ap)
            )
            nc.gpsimd.dma_start(out=bias_sb, in_=bias_bcast)

            for t in range(TT):
                htile = ht0 if t == 0 else load_htile(t)
                logits_ps = plpool.tile([P, n_experts], f32)

                n_mm = 0
                for b in range(KC // BATCH):
                    tp = ptpool.tile([P, BATCH * P], f32)
                    for j in range(BATCH):
                        k = b * BATCH + j
                        if scheme == "f32r1":
                            nc.tensor.transpose(
                                tp[:, j * P : (j + 1) * P].bitcast(f32r),
                                htile[:, k * P : (k + 1) * P].bitcast(f32r),
                                identity.bitcast(f32r),
                            )
                        else:
                            nc.tensor.transpose(
                                tp[:, j * P : (j + 1) * P],
                                htile[:, k * P : (k + 1) * P],
                                identity,
                            )
                    hT = htpool.tile([P, BATCH * P], f32 if scheme == "f32r1" else f16)
                    # copyback PSUM->SBUF alternates between the (otherwise
                    # idle) scalar engine and DVE
                    if b % 2 == 0:
                        nc.scalar.activation(
                            hT, tp, mybir.ActivationFunctionType.Copy
                        )
                    else:
                        nc.vector.tensor_copy(hT, tp)
                    for j in range(BATCH):
                        k = b * BATCH + j
                        if scheme == "f32r1":
                            lhsT = hT[:, j * P : (j + 1) * P].bitcast(f32r)
                            rhs = wk_mm[:, k, :].bitcast(f32r)
                        else:
                            lhsT = hT[:, j * P : (j + 1) * P]
                            rhs = wk_mm[:, k, :]
                        nc.tensor.matmul(
                            logits_ps,
                            lhsT=lhsT,
                            rhs=rhs,
                            start=(n_mm == 0),
                            stop=(n_mm == KC - 1),
                        )
                        n_mm += 1

                # ---- routing epilogue (tokens on partitions) ----
                sc = rpool.tile([P, n_experts], f32)
                nc.scalar.activation(
                    sc, logits_ps, mybir.ActivationFunctionType.Sigmoid
                )
                nc.vector.tensor_add(sc, sc, bias_sb)

                # top-2 sum per group of GS experts
                m8 = rpool.tile([P, N_GROUP * 8], f32)
                for g in range(N_GROUP):
                    nc.vector.max(
                        m8[:, g * 8 : (g + 1) * 8], sc[:, g * GS : (g + 1) * GS]
                    )
                m8v = m8.rearrange("p (g k) -> p g k", k=8)
                gsum = rpool.tile([P, N_GROUP], f32)
                nc.vector.tensor_add(gsum, m8v[:, :, 0], m8v[:, :, 1])

                # top-TOPK_GROUP groups -> per-group 0/1 mask via threshold
                gmax = rpool.tile([P, 8], f32)
                nc.vector.max(gmax, gsum)
                gmask = rpool.tile([P, N_GROUP], f32)
                nc.vector.tensor_scalar(
                    gmask,
                    gsum,
                    gmax[:, TOPK_GROUP - 1 : TOPK_GROUP],
                    None,
                    op0=mybir.AluOpType.is_ge,
                )

                # masked scores = sc * mask (0 where group dropped)
                masked = rpool.tile([P, n_experts], f32)
                nc.vector.tensor_mul(
                    masked.rearrange("p (g e) -> p g e", g=N_GROUP),
                    sc.rearrange("p (g e) -> p g e", g=N_GROUP),
                    gmask[:, :, None].broadcast_to([P, N_GROUP, GS]),
                )

                top8 = rpool.tile([P, TOP_K], f32)
                nc.vector.max(top8, masked)

                dsum = rpool.tile([P, 1], f32)
                nc.vector.reduce_sum(dsum, top8, axis=mybir.AxisListType.X)
                rcp = rpool.tile([P, 1], f32)
                nc.vector.reciprocal(rcp, dsum)
                wout = rpool.tile([P, TOP_K], f32)
                nc.vector.tensor_scalar(
                    wout,
                    top8,
                    rcp,
                    SCALE,
                    op0=mybir.AluOpType.mult,
                    op1=mybir.AluOpType.mult,
                )
                nc.sync.dma_start(out=out[t * P : (t + 1) * P, :], in_=wout)

    nc.compile()
    return nc


_CACHE = {}


def _built_nc():
    if "nc" not in _CACHE:
        _CACHE["nc"] = build_moe_gate()
    return _CACHE["nc"]


def kernel(hidden_states, kernel, e_score_correction_bias):
    hs = np.ascontiguousarray(np.asarray(hidden_states), dtype=np.float32)
    wk = np.ascontiguousarray(np.asarray(kernel), dtype=np.float32)
    bi = np.ascontiguousarray(np.asarray(e_score_correction_bias), dtype=np.float32)
    assert hs.shape == (TOKENS, HIDDEN) and wk.shape == (HIDDEN, EXPERTS)

    tpc = TOKENS // N_CORES
    nc = _built_nc()
    in_maps = [
        {
            "hidden_states": hs[i * tpc : (i + 1) * tpc],
            "kernel": wk,
            "e_score_correction_bias": bi,
        }
        for i in range(N_CORES)
    ]
    res = bass_utils.run_bass_kernel_spmd(nc, in_maps, core_ids=list(range(N_CORES)))
    return np.concatenate(
        [res.results[i]["topk_out"] for i in range(N_CORES)], axis=0
    )
